# revision 14
# baseline (speedup 1.0000x reference)
"""Bass/Trainium2 kernel for nn_DWAMiddleLayer (low-rank MoE weight-assembly layer).

Math (reference):
    U    = pool[:, :1024].reshape(N, DB, R)      # [512, 256, 4]
    V    = pool[:, 1024:2048].reshape(N, R, DA)  # [512, 4, 256]
    bE   = pool[:, 2048:2304]                    # [512, 256]
    h_t  = h_A @ W_base.T
           + sum_r (alpha * (h_A @ V_r.T)) @ U_r          # never materialize W_assembled
           + alpha @ bE + b_base
    y    = h_A + gamma * h_t ; out = LayerNorm(y) * ln_scale + ln_bias

Distribution: data-parallel over batch B=2048 across 8 cores (BS=256 rows each);
pool/W_base/vectors replicated. h_t is computed in transposed space (feature dim
on partitions, batch on the free dim) so that every matmul contraction dim lands
on partitions naturally; layout transposes are PE identity-matmuls in bf16.
All matmul operands are bf16 (the gamma=1e-2 residual scaling makes matmul
rounding error negligible in the output); pool chunks arrive as SWDGE cast-DMAs,
small operands arrive in one packed HWDGE load and are cast on DVE. The
residual + LayerNorm path uses the untransposed fp32 h_A directly.
"""

import numpy as np

B, N, D_A, D_B, R = 2048, 512, 256, 256, 4
NC_COUNT = 8
BS = B // NC_COUNT  # 256 batch rows per core
P = 128
LN_EPS = 1e-5
POOL_W = D_B * R + R * D_A + D_B  # 2304 used columns of pool_vectors
U_OFF, V_OFF, BE_OFF = 0, D_B * R, D_B * R + R * D_A

# packed "smalls" tensor layout (fp32 elements per partition)
PK_HA = 0  # [2, 256]
PK_WB = 512  # [2, 256]
PK_ID = 1024  # 128 bf16 = 64 fp32 words
PK_BB = 1088  # [256] on partition 0 only
PK_W = 1344
# epilogue constants tensor [P, 513]: lsc(256) lbi(256) gamma(1)
EP_W = 513

_cache = {}


def _build_nc():
    import concourse.mybir as mybir
    import concourse.tile as tile
    from concourse import bacc

    fp32 = mybir.dt.float32
    bf16 = mybir.dt.bfloat16

    nc = bacc.Bacc("TRN2", target_bir_lowering=False)

    # ---- DRAM I/O (per-core shard shapes) ----
    d_pk = nc.dram_tensor("packed", [P, PK_W], fp32, kind="ExternalInput")
    d_al = nc.dram_tensor("alpha", [BS, N], fp32, kind="ExternalInput")
    d_ep = nc.dram_tensor("epconst", [P, EP_W], fp32, kind="ExternalInput")
    d_VbE = nc.dram_tensor("VbEpool", [N, R * D_A + D_B], fp32, kind="ExternalInput")
    d_U = nc.dram_tensor("Upool", [N, D_B * R], fp32, kind="ExternalInput")
    d_out = nc.dram_tensor("out", [BS, D_A], fp32, kind="ExternalOutput")

    with tile.TileContext(nc) as tc:
        with (
            tc.tile_pool(name="persist", bufs=1) as persist,
            tc.tile_pool(name="stage", bufs=4) as stage,
            tc.tile_pool(name="sm", bufs=3) as sm,
            tc.tile_pool(name="pp_tr", bufs=3, space="PSUM") as pp_tr,
            tc.tile_pool(name="pp_t", bufs=2, space="PSUM") as pp_t,
            tc.tile_pool(name="pp_acc", bufs=1, space="PSUM") as pp_acc,
        ):
            # ---------- tiny constants ----------
            eps_col = persist.tile([P, 1], fp32)
            nc.vector.memset(eps_col, LN_EPS)
            ones_row = persist.tile([1, BS], bf16)
            nc.vector.memset(ones_row, 1.0)
            # warm the ACT Sqrt table so the LN tail doesn't pay ACT_TABLE_LOAD
            warm = sm.tile([P, 1], fp32, tag="warm")
            nc.scalar.activation(
                warm, eps_col, mybir.ActivationFunctionType.Sqrt, bias=eps_col
            )

            # ---------- loads ----------
            # small packed HWDGE DMA (lands first; sync queue otherwise idle)
            pk = persist.tile([P, PK_W], fp32)
            nc.sync.dma_start(pk, d_pk[:])
            hA_sb = pk[:, PK_HA : PK_HA + 512].rearrange("p (o a) -> p o a", o=2)
            ident_b = pk[:, PK_ID : PK_ID + 64].bitcast(bf16)
            bb_row = pk[0:1, PK_BB : PK_BB + 256]

            # alpha via SWDGE cast-DMA, ahead of the pool chunks
            alpha_bf = persist.tile([P, 2, N], bf16)
            nc.gpsimd.dma_start(
                alpha_bf, d_al[:].rearrange("(o p) n -> p o n", p=P)
            )
            # pool chunks via SWDGE cast-DMA (fp32 HBM read -> bf16 SBUF write);
            # V|bE slab first so the transpose chain starts before U lands
            VBE_W = R * D_A + D_B
            VbE = [
                stage.tile([P, VBE_W], bf16, tag="vbe", name=f"VbE{o}")
                for o in range(4)
            ]
            U_bf = [
                stage.tile([P, D_B * R], bf16, tag="ubf", name=f"Ubf{o}")
                for o in range(4)
            ]
            for o in range(4):
                nc.gpsimd.dma_start(VbE[o], d_VbE[o * P : (o + 1) * P, :])
                nc.gpsimd.dma_start(U_bf[o], d_U[o * P : (o + 1) * P, :])

            # epilogue constants (HWDGE, after the packed smalls)
            ep = persist.tile([P, EP_W], fp32)
            nc.sync.dma_start(ep, d_ep[:])
            lsc_row = ep[:, 0:256]
            lbi_row = ep[:, 256:512]
            gamma_col = ep[:, 512:513]

            # bf16 casts of the packed smalls (DVE)
            hA_bf = sm.tile([P, 2, D_A], bf16, tag="hAbf")
            nc.vector.tensor_copy(hA_bf, hA_sb)
            Wb_bf = sm.tile([P, 2, D_A], bf16, tag="wbbf")
            nc.vector.tensor_copy(
                Wb_bf, pk[:, PK_WB : PK_WB + 512].rearrange("p (o a) -> p o a", o=2)
            )
            bb_bf = persist.tile([1, D_B], bf16)
            nc.vector.tensor_copy(bb_bf, bb_row)

            # ---------- transposes of small operands (PE identity-matmul, bf16) ----------
            hAT_b = persist.tile([P, 2, BS], bf16)  # [p_a, a_chunk, b]
            for ach in range(2):
                ps = pp_tr.tile([P, 512], fp32, tag="tr")
                for bch in range(2):
                    nc.tensor.matmul(
                        ps[:, bch * P : (bch + 1) * P],
                        lhsT=hA_bf[:, bch, ach * P : (ach + 1) * P],
                        rhs=ident_b,
                        start=True,
                        stop=True,
                    )
                nc.any.tensor_copy(hAT_b[:, ach], ps[:, :BS])

            # alpha^T -> bf16 [p_n, n_chunk, b]
            alphaT_b = persist.tile([P, 4, BS], bf16)
            for och in range(4):
                ps = pp_tr.tile([P, 512], fp32, tag="tr")
                for bch in range(2):
                    nc.tensor.matmul(
                        ps[:, bch * P : (bch + 1) * P],
                        lhsT=alpha_bf[:, bch, och * P : (och + 1) * P],
                        rhs=ident_b,
                        start=True,
                        stop=True,
                    )
                nc.any.tensor_copy(alphaT_b[:, och], ps[:, :BS])

            # W_base^T -> bf16 [p_a, a_chunk, c]
            WbT_b = persist.tile([P, 2, D_B], bf16)
            for ach in range(2):
                ps = pp_tr.tile([P, 512], fp32, tag="tr")
                for cch in range(2):
                    nc.tensor.matmul(
                        ps[:, cch * P : (cch + 1) * P],
                        lhsT=Wb_bf[:, cch, ach * P : (ach + 1) * P],
                        rhs=ident_b,
                        start=True,
                        stop=True,
                    )
                nc.any.tensor_copy(WbT_b[:, ach], ps[:, :D_B])

            # ---------- h_t^T accumulator: 2 psum tiles [c_half, b] ----------
            htT = [
                pp_acc.tile([P, BS], fp32, tag=f"acc{ch}", name=f"htT{ch}")
                for ch in range(2)
            ]
            started = [False, False]

            def acc_mm(ch, lhsT, rhs, last=False):
                nc.tensor.matmul(
                    htT[ch],
                    lhsT=lhsT,
                    rhs=rhs,
                    start=(not started[ch]),
                    stop=last,
                    skip_group_check=True,
                )
                started[ch] = True

            # ---------- main pipeline over expert chunks (o = n//128) ----------
            # V layout per pool row: f = V_OFF + r*256 + a  (r-major)
            # U layout per pool row: f = c*4 + r            (c-major)
            VT_b = persist.tile([P, 2, 2048], bf16)  # [p_a, a_chunk, r*512+o*128+pn]
            U_bfr = persist.tile([P, 4, R, D_B], bf16)  # [p_n, o, r, c]

            for o in range(4):
                V_bf = VbE[o][:, 0 : R * D_A]
                # transpose V chunk: blocks (r, a_half) of [128n x 128a]
                for ach in range(2):
                    ps = pp_tr.tile([P, 512], fp32, tag="tr")
                    for r in range(4):
                        nc.tensor.matmul(
                            ps[:, r * P : (r + 1) * P],
                            lhsT=V_bf[:, r * D_A + ach * P : r * D_A + (ach + 1) * P],
                            rhs=ident_b,
                            start=True,
                            stop=True,
                        )
                    # scatter the 4 r-blocks into VT at [r*512 + o*128]
                    dst = VT_b[:, ach].rearrange("p (r q) -> p r q", r=4)[
                        :, :, o * P : (o + 1) * P
                    ]
                    nc.any.tensor_copy(dst, ps[:].rearrange("p (r q) -> p r q", r=4))

                # destride U chunk (c r) -> (r c) in bf16 on DVE
                nc.vector.tensor_copy(
                    U_bfr[:, o],
                    U_bf[o][:].rearrange("p (c r) -> p r c", r=R),
                )

                for rp in range(2):
                    # mm1 for an r-pair: t_r^T[n_chunk, b] = V_r @ h_A^T (contract a)
                    t_ps = pp_t.tile([P, 2, BS], fp32, tag="t")
                    for rr in range(2):
                        r = rp * 2 + rr
                        for ach in range(2):
                            nc.tensor.matmul(
                                t_ps[:, rr],
                                lhsT=VT_b[
                                    :, ach, r * 512 + o * P : r * 512 + (o + 1) * P
                                ],
                                rhs=hAT_b[:, ach],
                                start=(ach == 0),
                                stop=(ach == 1),
                            )
                    # s_r^T = alpha^T * t_r^T for both r's in one DVE op
                    s_bf = sm.tile([P, 2, BS], bf16, tag="s")
                    nc.vector.tensor_mul(
                        s_bf, t_ps, alphaT_b[:, o : o + 1, :].to_broadcast((P, 2, BS))
                    )
                    # mm2: h_t^T += U_r^T-chunks @ s_r^T (contract n)
                    for rr in range(2):
                        r = rp * 2 + rr
                        for ch in range(2):
                            acc_mm(
                                ch, U_bfr[:, o, r, ch * P : (ch + 1) * P], s_bf[:, rr]
                            )

                # bias-mm for this chunk: h_t^T += biasE^T @ alpha^T (contract n)
                bE_o = VbE[o][:, R * D_A : R * D_A + D_B]
                for ch in range(2):
                    acc_mm(
                        ch, bE_o[:, ch * P : (ch + 1) * P], alphaT_b[:, o], last=(o == 3)
                    )

                if o == 0:
                    # base-mm + b_base rank-1, folded in early (no DMA deps left)
                    for ch in range(2):
                        for ach in range(2):
                            acc_mm(
                                ch, WbT_b[:, ach, ch * P : (ch + 1) * P], hAT_b[:, ach]
                            )
                        acc_mm(ch, bb_bf[:, ch * P : (ch + 1) * P], ones_row)

            # ---------- epilogue: transpose h_t back, residual + LayerNorm in fp32 ----------
            htT_bf = sm.tile([P, 2, BS], bf16, tag="htTbf")
            for ch in range(2):
                nc.any.tensor_copy(htT_bf[:, ch], htT[ch])

            ht_ps = pp_tr.tile([P, 512], fp32, tag="tr", name="ht_ps")
            for bch in range(2):
                for jch in range(2):
                    nc.tensor.matmul(
                        ht_ps[:, bch * 256 + jch * P : bch * 256 + (jch + 1) * P],
                        lhsT=htT_bf[:, jch, bch * P : (bch + 1) * P],
                        rhs=ident_b,
                        start=True,
                        stop=True,
                        skip_group_check=True,
                    )

            out_sb = sm.tile([P, 2, D_A], fp32, tag="out")
            # y = h_A + gamma * h_t (fp32 residual), both b-chunks in one pass
            y_sb = sm.tile([P, 2, D_A], fp32, tag="y")
            nc.vector.scalar_tensor_tensor(
                y_sb,
                in0=ht_ps[:].rearrange("p (o a) -> p o a", o=2),
                scalar=gamma_col,
                in1=hA_sb,
                op0=mybir.AluOpType.mult,
                op1=mybir.AluOpType.add,
            )
            stats = sm.tile([P, 2, 6], fp32, tag="st")
            mv = sm.tile([P, 2, 2], fp32, tag="mv")
            for bch in range(2):
                nc.vector.bn_stats(stats[:, bch], y_sb[:, bch])
                nc.vector.bn_aggr(mv[:, bch], stats[:, bch])
            # rstd = 1/sqrt(var + eps) for both chunks at once
            rstd = sm.tile([P, 2], fp32, tag="rstd")
            nc.scalar.activation(
                rstd,
                mv[:, :, 1],
                mybir.ActivationFunctionType.Sqrt,
                bias=eps_col,
            )
            nc.vector.reciprocal(rstd, rstd)
            for bch in range(2):
                # (y - mu) * rstd
                nc.vector.tensor_scalar(
                    out_sb[:, bch],
                    y_sb[:, bch],
                    scalar1=mv[:, bch, 0:1],
                    scalar2=rstd[:, bch : bch + 1],
                    op0=mybir.AluOpType.subtract,
                    op1=mybir.AluOpType.mult,
                )
            # * ln_scale + ln_bias (both chunks, broadcast rows)
            nc.vector.tensor_mul(
                out_sb, out_sb, lsc_row.unsqueeze(1).to_broadcast((P, 2, D_A))
            )
            nc.vector.tensor_add(
                out_sb, out_sb, lbi_row.unsqueeze(1).to_broadcast((P, 2, D_A))
            )
            for bch in range(2):
                nc.sync.dma_start(
                    d_out[bch * P : (bch + 1) * P, :], out_sb[:, bch]
                )

    nc.compile()
    return nc


def _get_nc():
    if "nc" not in _cache:
        _cache["nc"] = _build_nc()
    return _cache["nc"]


def make_in_maps(**inputs):
    """Shard full inputs into 8 per-core input maps."""
    import ml_dtypes

    f32 = lambda x: np.ascontiguousarray(np.asarray(x), dtype=np.float32)
    h_A = f32(inputs["h_A"])
    alpha = f32(inputs["alpha"])
    pool = np.asarray(inputs["pool_vectors"], dtype=np.float32)
    Upool = np.ascontiguousarray(pool[:, : D_B * R])
    VbEpool = np.ascontiguousarray(pool[:, D_B * R : POOL_W])
    W_base = f32(inputs["W_base"])
    b_base = f32(inputs["b_base"]).reshape(D_B)
    gamma = float(np.asarray(inputs["gamma"]).reshape(()))
    ln_scale = f32(inputs["ln_scale"]).reshape(D_A)
    ln_bias = f32(inputs["ln_bias"]).reshape(D_A)

    ident = np.eye(P, dtype=np.float32).astype(ml_dtypes.bfloat16)
    ident_words = np.ascontiguousarray(ident).view(np.float32)  # [P, 64]

    ep = np.empty((P, EP_W), np.float32)
    ep[:, 0:256] = ln_scale[None, :]
    ep[:, 256:512] = ln_bias[None, :]
    ep[:, 512] = gamma

    wb_pk = np.ascontiguousarray(W_base.reshape(2, P, D_A).transpose(1, 0, 2)).reshape(
        P, 512
    )

    in_maps = []
    for i in range(NC_COUNT):
        sl = slice(i * BS, (i + 1) * BS)
        pk = np.zeros((P, PK_W), np.float32)
        pk[:, PK_HA : PK_HA + 512] = (
            h_A[sl].reshape(2, P, D_A).transpose(1, 0, 2).reshape(P, 512)
        )
        pk[:, PK_WB : PK_WB + 512] = wb_pk
        pk[:, PK_ID : PK_ID + 64] = ident_words
        pk[0, PK_BB : PK_BB + 256] = b_base
        in_maps.append(
            {
                "packed": pk,
                "alpha": np.ascontiguousarray(alpha[sl]),
                "epconst": ep,
                "Upool": Upool,
                "VbEpool": VbEpool,
            }
        )
    return in_maps


def run_kernel(trace=False, **inputs):
    from concourse.bass_utils import run_bass_kernel_spmd

    nc = _get_nc()
    in_maps = make_in_maps(**inputs)
    res = run_bass_kernel_spmd(nc, in_maps, core_ids=list(range(NC_COUNT)), trace=trace)
    out = np.concatenate([r["out"] for r in res.results], axis=0)
    return out.astype(np.float32), res


def kernel(**inputs) -> np.ndarray:
    out, _ = run_kernel(trace=False, **inputs)
    return out


# revision 16
# speedup vs baseline: 1.0076x; 1.0076x over previous
"""Bass/Trainium2 kernel for nn_DWAMiddleLayer (low-rank MoE weight-assembly layer).

Math (reference):
    U    = pool[:, :1024].reshape(N, DB, R)      # [512, 256, 4]
    V    = pool[:, 1024:2048].reshape(N, R, DA)  # [512, 4, 256]
    bE   = pool[:, 2048:2304]                    # [512, 256]
    h_t  = h_A @ W_base.T
           + sum_r (alpha * (h_A @ V_r.T)) @ U_r          # never materialize W_assembled
           + alpha @ bE + b_base
    y    = h_A + gamma * h_t ; out = LayerNorm(y) * ln_scale + ln_bias

Distribution: data-parallel over batch B=2048 across 8 cores (BS=256 rows each);
pool/W_base/vectors replicated. h_t is computed in transposed space (feature dim
on partitions, batch on the free dim) so that every matmul contraction dim lands
on partitions naturally; layout transposes are PE identity-matmuls in bf16.
All matmul operands are bf16 (the gamma=1e-2 residual scaling makes matmul
rounding error negligible in the output); pool chunks arrive as SWDGE cast-DMAs,
small operands arrive in one packed HWDGE load and are cast on DVE. The
residual + LayerNorm path uses the untransposed fp32 h_A directly.
"""

import numpy as np

B, N, D_A, D_B, R = 2048, 512, 256, 256, 4
NC_COUNT = 8
BS = B // NC_COUNT  # 256 batch rows per core
P = 128
LN_EPS = 1e-5
POOL_W = D_B * R + R * D_A + D_B  # 2304 used columns of pool_vectors
U_OFF, V_OFF, BE_OFF = 0, D_B * R, D_B * R + R * D_A

# packed "smalls" tensor layout (fp32 elements per partition)
PK_HA = 0  # [2, 256]
PK_WB = 512  # [2, 256]
PK_ID = 1024  # 128 bf16 = 64 fp32 words
PK_BB = 1088  # [256] on partition 0 only
PK_W = 1344
# epilogue constants tensor [P, 513]: lsc(256) lbi(256) gamma(1)
EP_W = 513

_cache = {}


def _build_nc():
    import concourse.mybir as mybir
    import concourse.tile as tile
    from concourse import bacc

    fp32 = mybir.dt.float32
    bf16 = mybir.dt.bfloat16

    nc = bacc.Bacc("TRN2", target_bir_lowering=False)

    # ---- DRAM I/O (per-core shard shapes) ----
    d_pk = nc.dram_tensor("packed", [P, PK_W], fp32, kind="ExternalInput")
    d_al = nc.dram_tensor("alpha", [BS, N], fp32, kind="ExternalInput")
    d_ep = nc.dram_tensor("epconst", [P, EP_W], fp32, kind="ExternalInput")
    d_UV = nc.dram_tensor("UVpool", [N, POOL_W], fp32, kind="ExternalInput")
    d_out = nc.dram_tensor("out", [BS, D_A], fp32, kind="ExternalOutput")

    with tile.TileContext(nc) as tc:
        with (
            tc.tile_pool(name="persist", bufs=1) as persist,
            tc.tile_pool(name="stage", bufs=4) as stage,
            tc.tile_pool(name="sm", bufs=3) as sm,
            tc.tile_pool(name="pp_tr", bufs=3, space="PSUM") as pp_tr,
            tc.tile_pool(name="pp_t", bufs=2, space="PSUM") as pp_t,
            tc.tile_pool(name="pp_acc", bufs=1, space="PSUM") as pp_acc,
        ):
            # ---------- tiny constants ----------
            eps_col = persist.tile([P, 1], fp32)
            nc.vector.memset(eps_col, LN_EPS)
            ones_row = persist.tile([1, BS], bf16)
            nc.vector.memset(ones_row, 1.0)
            # warm the ACT Sqrt table so the LN tail doesn't pay ACT_TABLE_LOAD
            warm = sm.tile([P, 1], fp32, tag="warm")
            nc.scalar.activation(
                warm, eps_col, mybir.ActivationFunctionType.Sqrt, bias=eps_col
            )

            # ---------- loads ----------
            # small packed HWDGE DMA (lands first; sync queue otherwise idle)
            pk = persist.tile([P, PK_W], fp32)
            nc.sync.dma_start(pk, d_pk[:])
            hA_sb = pk[:, PK_HA : PK_HA + 512].rearrange("p (o a) -> p o a", o=2)
            ident_b = pk[:, PK_ID : PK_ID + 64].bitcast(bf16)
            bb_row = pk[0:1, PK_BB : PK_BB + 256]

            # alpha via SWDGE cast-DMA, ahead of the pool chunks
            alpha_bf = persist.tile([P, 2, N], bf16)
            nc.gpsimd.dma_start(
                alpha_bf, d_al[:].rearrange("(o p) n -> p o n", p=P)
            )
            # pool chunks via SWDGE cast-DMA (fp32 HBM read -> bf16 SBUF write)
            UVc = [
                stage.tile([P, POOL_W], bf16, tag="uvc", name=f"UVc{o}")
                for o in range(4)
            ]
            for o in range(4):
                nc.gpsimd.dma_start(UVc[o], d_UV[o * P : (o + 1) * P, :])

            # epilogue constants (HWDGE, after the packed smalls)
            ep = persist.tile([P, EP_W], fp32)
            nc.sync.dma_start(ep, d_ep[:])
            lsc_row = ep[:, 0:256]
            lbi_row = ep[:, 256:512]
            gamma_col = ep[:, 512:513]

            # bf16 casts of the packed smalls (DVE)
            hA_bf = sm.tile([P, 2, D_A], bf16, tag="hAbf")
            nc.vector.tensor_copy(hA_bf, hA_sb)
            Wb_bf = sm.tile([P, 2, D_A], bf16, tag="wbbf")
            nc.vector.tensor_copy(
                Wb_bf, pk[:, PK_WB : PK_WB + 512].rearrange("p (o a) -> p o a", o=2)
            )
            bb_bf = persist.tile([1, D_B], bf16)
            nc.vector.tensor_copy(bb_bf, bb_row)

            # ---------- transposes of small operands (PE identity-matmul, bf16) ----------
            hAT_b = persist.tile([P, 2, BS], bf16)  # [p_a, a_chunk, b]
            for ach in range(2):
                ps = pp_tr.tile([P, 512], fp32, tag="tr")
                for bch in range(2):
                    nc.tensor.matmul(
                        ps[:, bch * P : (bch + 1) * P],
                        lhsT=hA_bf[:, bch, ach * P : (ach + 1) * P],
                        rhs=ident_b,
                        start=True,
                        stop=True,
                    )
                nc.any.tensor_copy(hAT_b[:, ach], ps[:, :BS])

            # alpha^T -> bf16 [p_n, n_chunk, b]
            alphaT_b = persist.tile([P, 4, BS], bf16)
            for och in range(4):
                ps = pp_tr.tile([P, 512], fp32, tag="tr")
                for bch in range(2):
                    nc.tensor.matmul(
                        ps[:, bch * P : (bch + 1) * P],
                        lhsT=alpha_bf[:, bch, och * P : (och + 1) * P],
                        rhs=ident_b,
                        start=True,
                        stop=True,
                    )
                nc.any.tensor_copy(alphaT_b[:, och], ps[:, :BS])

            # W_base^T -> bf16 [p_a, a_chunk, c]
            WbT_b = persist.tile([P, 2, D_B], bf16)
            for ach in range(2):
                ps = pp_tr.tile([P, 512], fp32, tag="tr")
                for cch in range(2):
                    nc.tensor.matmul(
                        ps[:, cch * P : (cch + 1) * P],
                        lhsT=Wb_bf[:, cch, ach * P : (ach + 1) * P],
                        rhs=ident_b,
                        start=True,
                        stop=True,
                    )
                nc.any.tensor_copy(WbT_b[:, ach], ps[:, :D_B])

            # ---------- h_t^T accumulator: 2 psum tiles [c_half, b] ----------
            htT = [
                pp_acc.tile([P, BS], fp32, tag=f"acc{ch}", name=f"htT{ch}")
                for ch in range(2)
            ]
            started = [False, False]

            def acc_mm(ch, lhsT, rhs, last=False):
                nc.tensor.matmul(
                    htT[ch],
                    lhsT=lhsT,
                    rhs=rhs,
                    start=(not started[ch]),
                    stop=last,
                    skip_group_check=True,
                )
                started[ch] = True

            # ---------- main pipeline over expert chunks (o = n//128) ----------
            # V layout per pool row: f = V_OFF + r*256 + a  (r-major)
            # U layout per pool row: f = c*4 + r            (c-major)
            VT_b = persist.tile([P, 2, 2048], bf16)  # [p_a, a_chunk, r*512+o*128+pn]
            U_bfr = persist.tile([P, 4, R, D_B], bf16)  # [p_n, o, r, c]

            for o in range(4):
                V_bf = UVc[o][:, V_OFF : V_OFF + R * D_A]
                # transpose V chunk: blocks (r, a_half) of [128n x 128a]
                for ach in range(2):
                    ps = pp_tr.tile([P, 512], fp32, tag="tr")
                    for r in range(4):
                        nc.tensor.matmul(
                            ps[:, r * P : (r + 1) * P],
                            lhsT=V_bf[:, r * D_A + ach * P : r * D_A + (ach + 1) * P],
                            rhs=ident_b,
                            start=True,
                            stop=True,
                        )
                    # scatter the 4 r-blocks into VT at [r*512 + o*128]
                    dst = VT_b[:, ach].rearrange("p (r q) -> p r q", r=4)[
                        :, :, o * P : (o + 1) * P
                    ]
                    nc.any.tensor_copy(dst, ps[:].rearrange("p (r q) -> p r q", r=4))

                # destride U chunk (c r) -> (r c) in bf16 on DVE
                nc.vector.tensor_copy(
                    U_bfr[:, o],
                    UVc[o][:, U_OFF : U_OFF + D_B * R].rearrange(
                        "p (c r) -> p r c", r=R
                    ),
                )

                for rp in range(2):
                    # mm1 for an r-pair: t_r^T[n_chunk, b] = V_r @ h_A^T (contract a)
                    t_ps = pp_t.tile([P, 2, BS], fp32, tag="t")
                    for rr in range(2):
                        r = rp * 2 + rr
                        for ach in range(2):
                            nc.tensor.matmul(
                                t_ps[:, rr],
                                lhsT=VT_b[
                                    :, ach, r * 512 + o * P : r * 512 + (o + 1) * P
                                ],
                                rhs=hAT_b[:, ach],
                                start=(ach == 0),
                                stop=(ach == 1),
                            )
                    # s_r^T = alpha^T * t_r^T for both r's in one DVE op
                    s_bf = sm.tile([P, 2, BS], bf16, tag="s")
                    nc.vector.tensor_mul(
                        s_bf, t_ps, alphaT_b[:, o : o + 1, :].to_broadcast((P, 2, BS))
                    )
                    # mm2: h_t^T += U_r^T-chunks @ s_r^T (contract n)
                    for rr in range(2):
                        r = rp * 2 + rr
                        for ch in range(2):
                            acc_mm(
                                ch, U_bfr[:, o, r, ch * P : (ch + 1) * P], s_bf[:, rr]
                            )

                # bias-mm for this chunk: h_t^T += biasE^T @ alpha^T (contract n)
                bE_o = UVc[o][:, BE_OFF : BE_OFF + D_B]
                for ch in range(2):
                    acc_mm(
                        ch, bE_o[:, ch * P : (ch + 1) * P], alphaT_b[:, o], last=(o == 3)
                    )

                if o == 0:
                    # base-mm + b_base rank-1, folded in early (no DMA deps left)
                    for ch in range(2):
                        for ach in range(2):
                            acc_mm(
                                ch, WbT_b[:, ach, ch * P : (ch + 1) * P], hAT_b[:, ach]
                            )
                        acc_mm(ch, bb_bf[:, ch * P : (ch + 1) * P], ones_row)

            # ---------- epilogue: transpose h_t back, residual + LayerNorm in fp32 ----------
            htT_bf = sm.tile([P, 2, BS], bf16, tag="htTbf")
            for ch in range(2):
                nc.any.tensor_copy(htT_bf[:, ch], htT[ch])

            ht_ps = pp_tr.tile([P, 512], fp32, tag="tr", name="ht_ps")
            for bch in range(2):
                for jch in range(2):
                    nc.tensor.matmul(
                        ht_ps[:, bch * 256 + jch * P : bch * 256 + (jch + 1) * P],
                        lhsT=htT_bf[:, jch, bch * P : (bch + 1) * P],
                        rhs=ident_b,
                        start=True,
                        stop=True,
                        skip_group_check=True,
                    )

            out_sb = sm.tile([P, 2, D_A], fp32, tag="out")
            # y = h_A + gamma * h_t (fp32 residual), both b-chunks in one pass
            y_sb = sm.tile([P, 2, D_A], fp32, tag="y")
            nc.vector.scalar_tensor_tensor(
                y_sb,
                in0=ht_ps[:].rearrange("p (o a) -> p o a", o=2),
                scalar=gamma_col,
                in1=hA_sb,
                op0=mybir.AluOpType.mult,
                op1=mybir.AluOpType.add,
            )
            stats = sm.tile([P, 2, 6], fp32, tag="st")
            mv = sm.tile([P, 2, 2], fp32, tag="mv")
            for bch in range(2):
                nc.vector.bn_stats(stats[:, bch], y_sb[:, bch])
                nc.vector.bn_aggr(mv[:, bch], stats[:, bch])
            # rstd = 1/sqrt(var + eps) for both chunks at once
            rstd = sm.tile([P, 2], fp32, tag="rstd")
            nc.scalar.activation(
                rstd,
                mv[:, :, 1],
                mybir.ActivationFunctionType.Sqrt,
                bias=eps_col,
            )
            nc.vector.reciprocal(rstd, rstd)
            for bch in range(2):
                # (y - mu) * rstd
                nc.vector.tensor_scalar(
                    out_sb[:, bch],
                    y_sb[:, bch],
                    scalar1=mv[:, bch, 0:1],
                    scalar2=rstd[:, bch : bch + 1],
                    op0=mybir.AluOpType.subtract,
                    op1=mybir.AluOpType.mult,
                )
            # * ln_scale + ln_bias (both chunks, broadcast rows)
            nc.vector.tensor_mul(
                out_sb, out_sb, lsc_row.unsqueeze(1).to_broadcast((P, 2, D_A))
            )
            nc.vector.tensor_add(
                out_sb, out_sb, lbi_row.unsqueeze(1).to_broadcast((P, 2, D_A))
            )
            for bch in range(2):
                nc.sync.dma_start(
                    d_out[bch * P : (bch + 1) * P, :], out_sb[:, bch]
                )

    nc.compile()
    return nc


def _get_nc():
    if "nc" not in _cache:
        _cache["nc"] = _build_nc()
    return _cache["nc"]


def make_in_maps(**inputs):
    """Shard full inputs into 8 per-core input maps."""
    import ml_dtypes

    f32 = lambda x: np.ascontiguousarray(np.asarray(x), dtype=np.float32)
    h_A = f32(inputs["h_A"])
    alpha = f32(inputs["alpha"])
    pool = np.asarray(inputs["pool_vectors"], dtype=np.float32)
    UVpool = np.ascontiguousarray(pool[:, :POOL_W])
    W_base = f32(inputs["W_base"])
    b_base = f32(inputs["b_base"]).reshape(D_B)
    gamma = float(np.asarray(inputs["gamma"]).reshape(()))
    ln_scale = f32(inputs["ln_scale"]).reshape(D_A)
    ln_bias = f32(inputs["ln_bias"]).reshape(D_A)

    ident = np.eye(P, dtype=np.float32).astype(ml_dtypes.bfloat16)
    ident_words = np.ascontiguousarray(ident).view(np.float32)  # [P, 64]

    ep = np.empty((P, EP_W), np.float32)
    ep[:, 0:256] = ln_scale[None, :]
    ep[:, 256:512] = ln_bias[None, :]
    ep[:, 512] = gamma

    wb_pk = np.ascontiguousarray(W_base.reshape(2, P, D_A).transpose(1, 0, 2)).reshape(
        P, 512
    )

    in_maps = []
    for i in range(NC_COUNT):
        sl = slice(i * BS, (i + 1) * BS)
        pk = np.zeros((P, PK_W), np.float32)
        pk[:, PK_HA : PK_HA + 512] = (
            h_A[sl].reshape(2, P, D_A).transpose(1, 0, 2).reshape(P, 512)
        )
        pk[:, PK_WB : PK_WB + 512] = wb_pk
        pk[:, PK_ID : PK_ID + 64] = ident_words
        pk[0, PK_BB : PK_BB + 256] = b_base
        in_maps.append(
            {
                "packed": pk,
                "alpha": np.ascontiguousarray(alpha[sl]),
                "epconst": ep,
                "UVpool": UVpool,
            }
        )
    return in_maps


def run_kernel(trace=False, **inputs):
    from concourse.bass_utils import run_bass_kernel_spmd

    nc = _get_nc()
    in_maps = make_in_maps(**inputs)
    res = run_bass_kernel_spmd(nc, in_maps, core_ids=list(range(NC_COUNT)), trace=trace)
    out = np.concatenate([r["out"] for r in res.results], axis=0)
    return out.astype(np.float32), res


def kernel(**inputs) -> np.ndarray:
    out, _ = run_kernel(trace=False, **inputs)
    return out
